# revision 3
# baseline (speedup 1.0000x reference)
"""Multi-head attention (B=2, S=2048, D=768, H=12) on 8 Trainium2 cores.

Sharding: core c handles batch b=c//4 and heads 3*(c%4)..3*(c%4)+3.
QKV weights column-sharded, Wo row-sharded (Megatron); host sums the 4
partial outputs per batch and adds bo.

v3 schedule (per core), changes over v2:
  - wq/wk widened to 256 cols with head2 columns duplicated at 192:256,
    so the head2 projection matmul emits [128,512] with the partition-dup
    built in - no gpsimd dup DMA on the qt_b/kt_b critical path.
  - wv in bf16 (host-converted): V-proj moving operand is [128,192]
    bf16 (1 cyc/row without the fp32r >=256 padding), output evacuated
    to k-major bf16 V with interleaved ones columns.
  - lead-in DMA order interleaves weight loads with the x stream in
    first-use order (wq,bq / xq0 / wk,bk / xk0 / wv,bv / xk123 / wo)
    so the first projection starts ~2us in instead of ~8us.
  - normalize: emit all 3 reciprocals, then all 3 gpsimd broadcasts,
    then all 3 multiplies, so DVE never head-of-line blocks on an
    in-flight broadcast; head1's partition shift rides the scalar
    (ACT) HWDGE queue, off both the sync stream and gpsimd.
  - scores/PV/oproj unchanged: 64-row score matmuls pair heads 0/1 in
    the upper/lower PE row groups; PV rides full-height 128-contraction
    matmuls with the softmax denominator in an interleaved ones column.
"""

import sys

sys.path.insert(0, "/opt/trn_rl_repo")

from contextlib import ExitStack

import numpy as np

import concourse.bacc as bacc
import concourse.bass as bass
import concourse.tile as tile
from concourse import mybir
from concourse.bass_utils import run_bass_kernel_spmd

F32 = mybir.dt.float32
F32R = mybir.dt.float32r
BF16 = mybir.dt.bfloat16

S = 2048  # sequence length
D = 768  # model dim
HP = 3  # heads per core
DK = 64  # head dim
DO = HP * DK  # 192 out-cols per core
KT = D // 128  # 6 contraction tiles for projections
NB = S // 512  # 4 sequence blocks of 512
NKT = S // 128  # 16 kpos tiles
G = NKT // 2  # 8 groups of 2 kpos tiles
VW = HP * 65 + 1  # 196: [1|V0|1|V1|1|V2|1] ones interleaved
EXP = mybir.ActivationFunctionType.Exp


def emit_kernel(nc, tc, t, reps=1, hoist=True):
    ctx = ExitStack()
    sb = ctx.enter_context(tc.tile_pool(name="sb", bufs=1))
    xa = ctx.enter_context(tc.tile_pool(name="xa", bufs=1))  # xk then xq
    xb = ctx.enter_context(tc.tile_pool(name="xb", bufs=1))  # xv
    ptp = ctx.enter_context(tc.tile_pool(name="ptp", bufs=1))
    work = ctx.enter_context(tc.tile_pool(name="work", bufs=2))
    ps = ctx.enter_context(tc.tile_pool(name="ps", bufs=1, space=bass.MemorySpace.PSUM))

    # ---- persistent SBUF tensors ----
    wq_sb = sb.tile([128, KT, 256], BF16)  # cols 192:256 = head2 dup
    wk_sb = sb.tile([128, KT, 256], BF16)
    wv_sb = sb.tile([128, KT, DO], BF16)
    wo1_sb = sb.tile([128, D], F32R)  # Wo rows 0:128
    wo2b_sb = sb.tile([128, D], F32R)  # rows 0:64 = Wo[128:192], 64:128 zero
    bq_sb = sb.tile([128, 2], F32)
    bk_sb = sb.tile([128, 2], F32)
    bv_bc = sb.tile([128, DO], F32)  # bv broadcast to 128 partitions
    qt_a = sb.tile([128, S], BF16)  # Q^T rows 0:128 (heads 0,1)
    qt_b = sb.tile([128, S], BF16)  # head 2 at 0:64, dup at 64:128
    kt_a = sb.tile([128, S], BF16)
    kt_b = sb.tile([128, S], BF16)
    v_sb = sb.tile([128, NKT, VW], BF16)  # k-major V with ones cols
    out_a = sb.tile([128, S], F32R)  # normalized attn out^T, heads 0,1
    out_b2 = sb.tile([128, S], F32R)  # head 2 at rows 0:64, rows 64:128 zero

    def wload(dst, w):
        nc.sync.dma_start(dst[:, :, 0:DO], w.ap().rearrange("(k p) o -> p k o", p=128))

    def wload_dup(dst, w):
        nc.sync.dma_start(
            dst[:, :, DO : DO + 64],
            w.ap().rearrange("(k p) o -> p k o", p=128)[:, :, 128:DO],
        )

    def bload(dst, b):
        nc.sync.dma_start(dst[:, 0:1], b.ap()[0:128, :])
        nc.sync.dma_start(dst[0:64, 1:2], b.ap()[128:DO, :])
        nc.sync.dma_start(dst[64:128, 1:2], b.ap()[128:DO, :])

    # ---- cheap inits (engines idle at t=0) ----
    nc.vector.memset(wo2b_sb[64:128, :].bitcast(F32), 0.0)
    nc.vector.memset(out_b2[64:128, :].bitcast(F32), 0.0)
    for oc in (0, 65, 130, 195):
        nc.vector.memset(v_sb[:, :, oc : oc + 1], 1.0)

    def dma_x_nb(xdram, pool, pfx, nb):
        """Load x[:, nb*512:(nb+1)*512] as 3 [128,2,512] bf16 tiles (paired
        k-chunks per DMA: halves descriptor/issue count)."""
        xts = []
        for j in range(KT // 2):
            xt = pool.tile(
                [128, 2, 512], BF16, name=f"{pfx}{j}", tag=f"{pfx}{j}", bufs=2
            )
            nc.sync.dma_start(
                xt[:],
                xdram.ap()[
                    j * 256 : j * 256 + 256, nb * 512 : nb * 512 + 512
                ].rearrange("(kk p) s -> p kk s", p=128),
            )
            xts.append(xt[:, 0, :])
            xts.append(xt[:, 1, :])
        return xts

    def proj_qk_nb(xts, w_sb, b_sb, dst_a, dst_b, nb):
        nb0 = nb * 512
        pq = ps.tile([128, 2, 512], F32, tag="ss", bufs=2, name="pq")
        p1, p2 = pq[:, 0, :], pq[:, 1, :]
        for k in range(KT):
            nc.tensor.matmul(
                p1, w_sb[:, k, 0:128], xts[k],
                start=(k == 0), stop=(k == KT - 1),
            )
        nc.vector.tensor_scalar_add(
            dst_a[:, nb0 : nb0 + 512], p1, b_sb[:, 0:1]
        )
        for k in range(KT):
            nc.tensor.matmul(
                p2, w_sb[:, k, 128:256], xts[k],
                start=(k == 0), stop=(k == KT - 1),
            )
        nc.vector.tensor_scalar_add(
            dst_b[:, nb0 : nb0 + 512], p2, b_sb[:, 1:2]
        )

    def proj_v_nb(xts, nb):
        # tag "py" (not "ss") so the V pipeline never waits on score-tile
        # exp frees; bufs=1 serializes st on the previous evac, which hides
        # under the exp stream anyway.
        for sl in range(4):
            st = nb * 4 + sl
            pv = ps.tile([128, DO], F32, tag="py", bufs=1, name="pv")
            for k in range(KT):
                nc.tensor.matmul(
                    pv[:], xts[k][:, sl * 128 : sl * 128 + 128],
                    wv_sb[:, k, :], start=(k == 0), stop=(k == KT - 1),
                )
            dst = v_sb[:, st, 1:196].rearrange("p (h c) -> p h c", h=HP)[:, :, 0:64]
            nc.vector.tensor_add(
                dst,
                pv[:].rearrange("p (h c) -> p h c", h=HP),
                bv_bc[:].rearrange("p (h c) -> p h c", h=HP),
            )

    def vslice(kt_i, h):
        return v_sb[:, kt_i, 1 + h * 65 : 1 + h * 65 + 65]

    def scores_qb(qb):
        """All 3 heads' S^T for q-block qb; exp into bf16 P^T tiles.

        Returns pts[g] = (ptx, pty, ptz): ptx[:,j,:] = P^T of head j for
        kpos chunk 2g; pty for chunk 2g+1; ptz[:,0,:]/[:,1,:] = head 2
        chunks 2g / 2g+1."""
        q0 = qb * 512
        pts = []
        for g in range(G):
            pts.append(score_group(q0, g))
        return pts

    def score_group(q0, g):
        ke, ko = 2 * g * 128, (2 * g + 1) * 128
        ssx = ps.tile([128, 2, 512], F32, tag="ss", bufs=2, name="ssx")
        nc.tensor.matmul(
            ssx[:, 0, :], kt_a[0:64, ke : ke + 128], qt_a[0:64, q0 : q0 + 512],
            start=True, stop=True,
        )
        nc.tensor.matmul(
            ssx[:, 1, :], kt_a[64:128, ke : ke + 128], qt_a[64:128, q0 : q0 + 512],
            start=True, stop=True,
        )
        ptx = ptp.tile([128, 2, 512], BF16, name="ptx", tag="pt", bufs=20)
        nc.scalar.activation(ptx[:], ssx[:], EXP, scale=0.125)
        ssy = ps.tile([128, 2, 512], F32, tag="ss", bufs=2, name="ssy")
        nc.tensor.matmul(
            ssy[:, 0, :], kt_a[0:64, ko : ko + 128], qt_a[0:64, q0 : q0 + 512],
            start=True, stop=True,
        )
        nc.tensor.matmul(
            ssy[:, 1, :], kt_a[64:128, ko : ko + 128], qt_a[64:128, q0 : q0 + 512],
            start=True, stop=True,
        )
        pty = ptp.tile([128, 2, 512], BF16, name="pty", tag="pt", bufs=20)
        nc.scalar.activation(pty[:], ssy[:], EXP, scale=0.125)
        ssz = ps.tile([128, 2, 512], F32, tag="ss", bufs=2, name="ssz")
        nc.tensor.matmul(
            ssz[:, 0, :], kt_b[0:64, ke : ke + 128], qt_b[0:64, q0 : q0 + 512],
            start=True, stop=True,
        )
        nc.tensor.matmul(
            ssz[:, 1, :], kt_b[64:128, ko : ko + 128], qt_b[64:128, q0 : q0 + 512],
            start=True, stop=True,
        )
        ptz = ptp.tile([128, 2, 512], BF16, name="ptz", tag="pt", bufs=20)
        nc.scalar.activation(ptz[:], ssz[:], EXP, scale=0.125)
        return (ptx, pty, ptz)

    def pv_qb(pts):
        """P^T @ [V|1] accumulated over kpos; returns po per head."""
        po0 = ps.tile([65, 512], F32, tag="po0", bufs=1, name="po0")
        po1 = ps.tile([65, 512], F32, tag="po1", bufs=1, name="po1")
        po2 = ps.tile([65, 512], F32, tag="po2", bufs=1, name="po2")
        for g in range(G):
            ptx, pty, ptz = pts[g]
            ke, ko = 2 * g, 2 * g + 1
            st, sp = g == 0, g == G - 1
            nc.tensor.matmul(po0[:], vslice(ke, 0), ptx[:, 0, :],
                             start=st, stop=False, skip_group_check=True)
            nc.tensor.matmul(po0[:], vslice(ko, 0), pty[:, 0, :],
                             start=False, stop=sp, skip_group_check=True)
            nc.tensor.matmul(po1[:], vslice(ke, 1), ptx[:, 1, :],
                             start=st, stop=False, skip_group_check=True)
            nc.tensor.matmul(po1[:], vslice(ko, 1), pty[:, 1, :],
                             start=False, stop=sp, skip_group_check=True)
            nc.tensor.matmul(po2[:], vslice(ke, 2), ptz[:, 0, :],
                             start=st, stop=False, skip_group_check=True)
            nc.tensor.matmul(po2[:], vslice(ko, 2), ptz[:, 1, :],
                             start=False, stop=sp, skip_group_check=True)
        return po0, po1, po2

    def normalize_qb(qb, po0, po1, po2):
        q0 = qb * 512
        pos = (po0, po1, po2)
        dts, dbs = [], []
        for h in range(HP):
            dtmp = work.tile([65, 512], F32, name=f"dt{h}", tag=f"dt{h}")
            nc.vector.reciprocal(dtmp[64:65, :], pos[h][64:65, :])
            dts.append(dtmp)
        for h in range(HP):
            dbc = work.tile([64, 512], F32, name=f"db{h}", tag=f"db{h}")
            nc.gpsimd.dma_start(
                dbc[:], dts[h][64:65, :].unsqueeze(1).to_broadcast([1, 64, 512])
            )
            dbs.append(dbc)
        nc.vector.tensor_mul(out_a[0:64, q0 : q0 + 512], po0[0:64, :], dbs[0][:])
        nsb = work.tile([64, 512], F32R, name="nsb", tag="nsb")
        nc.vector.tensor_mul(nsb[:], po1[0:64, :], dbs[1][:])
        nc.scalar.dma_start(out_a[64:128, q0 : q0 + 512], nsb[:])
        nc.vector.tensor_mul(out_b2[0:64, q0 : q0 + 512], po2[0:64, :], dbs[2][:])

    def oproj_qb(qb):
        q0 = qb * 512
        for ss in range(4):
            s0 = q0 + ss * 128
            ysb = work.tile([128, D], F32, name="ysb", tag="ysb", bufs=3)
            for half in range(2):
                c0 = half * 384
                py = ps.tile([128, 384], F32, tag=("py", "po0")[half],
                             bufs=1, name="py")
                nc.tensor.matmul(
                    py[:], out_a[:, s0 : s0 + 128], wo1_sb[:, c0 : c0 + 384],
                    start=True, stop=False,
                )
                nc.tensor.matmul(
                    py[:], out_b2[:, s0 : s0 + 128], wo2b_sb[:, c0 : c0 + 384],
                    start=False, stop=True,
                )
                nc.vector.tensor_copy(ysb[:, c0 : c0 + 384], py[:])
            nc.gpsimd.dma_start(t["y"].ap()[s0 : s0 + 128, :], ysb[:])

    def lead_in_scores0(first=False):
        """Lead-in with qb0 score groups interleaved per projected K block,
        so the exp stream starts as soon as kt(nb0) and qt(b0) exist.

        On the first rep the one-time weight loads are interleaved with the
        x stream in first-use order so the sync queue never front-loads
        cold weights ahead of the x tiles the PE is about to need."""
        if first:
            wload(wq_sb, t["wq"])
            wload_dup(wq_sb, t["wq"])
            bload(bq_sb, t["bq"])
        xts = dma_x_nb(t["xq"], xa, "a", 0)
        if first:
            wload(wk_sb, t["wk"])
            wload_dup(wk_sb, t["wk"])
            bload(bk_sb, t["bk"])
        xks = [dma_x_nb(t["xk"], xa, "a", nb) for nb in range(NB)]
        if first:
            nc.sync.dma_start(
                wv_sb[:], t["wv"].ap().rearrange("(k p) o -> p k o", p=128)
            )
            nc.sync.dma_start(
                bv_bc[:], t["bv"].ap().unsqueeze(1).to_broadcast([1, 128, DO])
            )
            nc.sync.dma_start(wo1_sb[:], t["wo"].ap()[0:128, :])
            nc.sync.dma_start(wo2b_sb[0:64, :], t["wo"].ap()[128:DO, :])
        proj_qk_nb(xts, wq_sb, bq_sb, qt_a, qt_b, 0)
        pts = []
        for nb in range(NB):
            proj_qk_nb(xks[nb], wk_sb, bk_sb, kt_a, kt_b, nb)
            pts.append(score_group(0, 2 * nb))
            pts.append(score_group(0, 2 * nb + 1))
        return pts

    pts0 = lead_in_scores0(first=True)
    for rep in range(reps):
        if rep > 0:
            pts0 = lead_in_scores0()

        for nb in range(2):
            xts = dma_x_nb(t["xv"], xb, "v", nb)
            proj_v_nb(xts, nb)
        xq1 = dma_x_nb(t["xq"], xa, "a", 1)
        xq2 = dma_x_nb(t["xq"], xa, "a", 2)
        for nb in range(2, NB):
            xts = dma_x_nb(t["xv"], xb, "v", nb)
            proj_v_nb(xts, nb)
        proj_qk_nb(xq1, wq_sb, bq_sb, qt_a, qt_b, 1)
        proj_qk_nb(xq2, wq_sb, bq_sb, qt_a, qt_b, 2)

        po = pv_qb(pts0)
        normalize_qb(0, *po)

        pts1 = scores_qb(1)
        oproj_qb(0)
        po = pv_qb(pts1)
        normalize_qb(1, *po)

        pts2 = scores_qb(2)
        oproj_qb(1)
        xts = dma_x_nb(t["xq"], xa, "a", 3)
        proj_qk_nb(xts, wq_sb, bq_sb, qt_a, qt_b, 3)
        po = pv_qb(pts2)
        normalize_qb(2, *po)

        pts3 = scores_qb(3)
        oproj_qb(2)
        po = pv_qb(pts3)
        normalize_qb(3, *po)
        oproj_qb(3)

    ctx.close()


_NC_CACHE = {}


def build_nc(reps=1, hoist=False):
    key = (reps, hoist)
    if key in _NC_CACHE:
        return _NC_CACHE[key]
    nc = bacc.Bacc("TRN2", target_bir_lowering=False, debug=False, num_devices=8)
    t = {}
    for name in ("xq", "xk", "xv"):
        t[name] = nc.dram_tensor(name, [D, S], BF16, kind="ExternalInput")
    for name in ("wq", "wk", "wv"):
        t[name] = nc.dram_tensor(name, [D, DO], BF16, kind="ExternalInput")
    t["wo"] = nc.dram_tensor("wo", [DO, D], F32R, kind="ExternalInput")
    for name in ("bq", "bk"):
        t[name] = nc.dram_tensor(name, [DO, 1], F32, kind="ExternalInput")
    t["bv"] = nc.dram_tensor("bv", [1, DO], F32, kind="ExternalInput")
    t["y"] = nc.dram_tensor("y", [S, D], F32, kind="ExternalOutput")

    with tile.TileContext(nc) as tc:
        emit_kernel(nc, tc, t, reps=reps, hoist=hoist)
    nc.compile()
    _NC_CACHE[key] = nc
    return nc


def make_in_maps(q, k, v, Wq, bq, Wk, bk, Wv, bv, Wo, bo):
    import ml_dtypes

    in_maps = []
    for c in range(8):
        b = c // 4
        hs = (c % 4) * DO
        in_maps.append(
            {
                "xq": np.ascontiguousarray(q[b].T.astype(ml_dtypes.bfloat16)),
                "xk": np.ascontiguousarray(k[b].T.astype(ml_dtypes.bfloat16)),
                "xv": np.ascontiguousarray(v[b].T.astype(ml_dtypes.bfloat16)),
                "wq": np.ascontiguousarray(
                    Wq[:, hs : hs + DO].astype(ml_dtypes.bfloat16)
                ),
                "wk": np.ascontiguousarray(
                    Wk[:, hs : hs + DO].astype(ml_dtypes.bfloat16)
                ),
                "wv": np.ascontiguousarray(
                    Wv[:, hs : hs + DO].astype(ml_dtypes.bfloat16)
                ),
                "wo": np.ascontiguousarray(Wo[hs : hs + DO, :]),
                "bq": np.ascontiguousarray(bq[hs : hs + DO, None]),
                "bk": np.ascontiguousarray(bk[hs : hs + DO, None]),
                "bv": np.ascontiguousarray(bv[None, hs : hs + DO]),
            }
        )
    return in_maps


def kernel(q, k, v, Wq, bq, Wk, bk, Wv, bv, Wo, bo, _reps=1):
    q = np.asarray(q, dtype=np.float32)
    k = np.asarray(k, dtype=np.float32)
    v = np.asarray(v, dtype=np.float32)
    nc = build_nc(reps=_reps)
    in_maps = make_in_maps(q, k, v, np.asarray(Wq), np.asarray(bq), np.asarray(Wk),
                           np.asarray(bk), np.asarray(Wv), np.asarray(bv),
                           np.asarray(Wo), np.asarray(bo))
    res = run_bass_kernel_spmd(nc, in_maps, list(range(8)))
    B = q.shape[0]
    y = np.zeros((B, S, D), dtype=np.float32)
    for c in range(8):
        y[c // 4] += res.results[c]["y"]
    y += np.asarray(bo, dtype=np.float32)[None, None, :]
    return y
